# revision 5
# baseline (speedup 1.0000x reference)
"""Trainium2 Bass kernel for nn_CompressiveMemory_57750130262084.

The reference computes (B=8, S=4096, DK=DV=1024):
    sigma  = elu(query) + 1                                  [B,S,DK]
    memory = einsum('bkd,bsv->bkv', swap(sigma), value)      [B,DK,DV]
    z_norm = sum_s sigma                                     [B,DK]
    out    = einsum('bsd,bkv->bsv', sigma, memory)
           / einsum('bsd,bk->bs',  sigma, z_norm)[..., None]

Every einsum uses disjoint summed subscripts, so each factorises into
outer products of independent reductions; everything cancels except
    out[b,s,v] = sum_s value[b,s,v]     (exactly; query cancels)

So the kernel is a column-sum of `value` over S, broadcast over S.
Sharding: data-parallel over batch, one NeuronCore per batch element.
Per-core work: read 16 MB, reduce 4096 rows -> 1 row, write 16 MB.

Measured facts driving this schedule (from NTFF traces on this pod):
  - per-NC DMA sustains ~405-414 GB/s aggregate; each of the 16 SDMA
    engines is ~line-rate (4 KB packet = ~146 ns read / ~162 ns write)
    and ~100%% busy in both phases, EXCEPT engine 15 which is ~14%%
    slower (166/185 ns) and straggles each phase by ~6 us.
  - f32 matmul = 2 HW passes per instruction: a [128,1024] chunk costs
    ~1.7 us on the PE but only ~1.23 us on the DVE; chunk line rate is
    1.25 us, so the DVE is the only engine that can keep pace alone.
  - DMA completion semaphores fire ~2.5 us after the last byte
    (HBM receipt round-trip), so per-chunk consumers always trail the
    stream by one DMA + 2.5 us: keep every DMA small (512 KB).

Schedule per core:
  - Row->partition assignment is arbitrary for a full sum (and all
    output rows are identical), so partitions 92-95/124-127 (the ones
    SDMA engine 15 serves) get only 28 rows while fast partitions get
    32-33: engine 15 finishes with the pack instead of +6 us late.
    Layout: 28 full [128,1024] slots (rows 0..3583) + remainder slots
    on fast partitions only: 4x[92 rows on p0-91], 5x[28 on p96-123],
    1x[4 on p0-3].
  - DVE accumulates 27 of the full slots (tensor_copy + 26 adds); the
    PE (idle otherwise) reduces the remainder slots straight into PSUM
    with partial-partition ones^T matmuls, folds the DVE accumulator
    in after its last add, and takes the final full slot directly so
    the tail after the last input byte is sem + 4 PE passes + copy.
  - PSUM -> SBUF copy in halves (DVE + ACT in parallel); ACT table is
    preloaded by a dummy scalar.copy at t=0 (lazy load costs 1.3 us).
  - writes are issued on the SCALAR HWDGE queue with a step-0
    broadcast source AP, mirroring the same slow-engine row split.
"""

import numpy as np

B, S, D = 8, 4096, 1024
P = 128                 # SBUF partitions
H = 512                 # PSUM bank width in f32 (matmul N limit)

# Row layout: engine 15 serves partitions 92-95 and 124-127; give them
# fewer rows.  28 full-width slots + remainder on fast partitions.
N_FULL = 28                       # [128, 1024] slots, rows 0..3583
B1_P, B1_N = 92, 4                # partitions 0..91,  rows 3584..3951
B2_P0, B2_P1, B2_N = 64, 92, 5    # partitions 64..91,  rows 3952..4091
# (odd engines 1-13 serve partitions 64-91 AND 96-123; base partition
#  must be 0/32/64 for matmul operands, so use the 64-91 block)
C_P, C_N = 4, 1                   # partitions 0..3,    rows 4092..4095
assert N_FULL * P + B1_P * B1_N + (B2_P1 - B2_P0) * B2_N + C_P * C_N == S

W_REP = 7                         # full-slot reps per write DMA
N_WA = N_FULL // W_REP            # 4 write DMAs for the full slots

_CACHE: dict = {}


def _build_program():
    import concourse.mybir as mybir
    import concourse.tile as tile
    from concourse import bacc

    f32 = mybir.dt.float32
    nc = bacc.Bacc("TRN2", target_bir_lowering=False, debug=False, num_devices=B, enable_asserts=False)
    v = nc.declare_dram_parameter("value", [S, D], f32, isOutput=False)
    o = nc.declare_dram_parameter("out", [S, D], f32, isOutput=True)

    vf = v[:]                                      # [4096, 1024]
    of = o[:]

    with tile.TileContext(nc) as tc:
        with (
            tc.tile_pool(name="in", bufs=1) as in_pool,
            tc.tile_pool(name="acc", bufs=1) as acc_pool,
            tc.tile_pool(name="ones", bufs=1) as ones_pool,
            tc.tile_pool(name="bcast", bufs=1) as bcast_pool,
            tc.tile_pool(name="warm", bufs=1) as warm_pool,
            tc.tile_pool(name="psum", bufs=1, space="PSUM") as psum_pool,
        ):
            # Preload the ACT table so the tail-time scalar.copy is cheap.
            warm = warm_pool.tile([P, 2], f32)
            nc.vector.memset(warm[:], 0.0)
            nc.scalar.copy(warm[:, 0:1], warm[:, 1:2])

            ones = ones_pool.tile([P, P], f32)
            nc.vector.memset(ones[:], 1.0)

            # ---- input DMAs (sync HWDGE queue), one 512 KB DMA per full
            # slot; remainder DMAs interleaved mid-stream.
            full_t = []
            for k in range(N_FULL):
                t = in_pool.tile([P, D], f32, tag=f"a{k}")
                nc.sync.dma_start(t[:], vf[k * P : (k + 1) * P])
                full_t.append(t)
                if k == 1:
                    tb1 = in_pool.tile([P, B1_N * D], f32, tag="b1")
                    src = vf[N_FULL * P : N_FULL * P + B1_N * B1_P].rearrange(
                        "(n p) m -> p n m", p=B1_P
                    )
                    nc.sync.dma_start(
                        tb1[0:B1_P].rearrange("p (n m) -> p n m", n=B1_N), src
                    )
                if k == 12:
                    nb2 = B2_P1 - B2_P0
                    r0 = N_FULL * P + B1_N * B1_P
                    tb2 = in_pool.tile([P, B2_N * D], f32, tag="b2")
                    src = vf[r0 : r0 + B2_N * nb2].rearrange("(n p) m -> p n m", p=nb2)
                    nc.sync.dma_start(
                        tb2[B2_P0:B2_P1].rearrange("p (n m) -> p n m", n=B2_N), src
                    )
                if k == 22:
                    tc_ = in_pool.tile([P, D], f32, tag="c")
                    nc.sync.dma_start(tc_[0:C_P], vf[S - C_P : S])

            # ---- reduction
            ps = psum_pool.tile([P, D], f32)

            def mm(moving, p0, p1, start, stop):
                for h in range(2):
                    nc.tensor.matmul(
                        ps[:, h * H : (h + 1) * H],
                        ones[p0:p1],
                        moving[p0:p1, h * H : (h + 1) * H],
                        start=start,
                        stop=stop,
                    )

            # PE: remainder slots straight into PSUM (plenty of slack).
            for n in range(B1_N):
                mm(tb1[:, n * D : (n + 1) * D], 0, B1_P, start=(n == 0), stop=False)
            for n in range(B2_N):
                mm(tb2[:, n * D : (n + 1) * D], B2_P0, B2_P1, start=False, stop=False)

            # DVE: full slots 0..26 into acc; the 4-row c slot is folded
            # by the DVE too (a contract-dim-4 matmul faults the PE:
            # NRT_EXEC_UNIT_UNRECOVERABLE, bisected on HW).
            acc = acc_pool.tile([P, D], f32)
            nc.vector.tensor_copy(acc[:], full_t[0][:])
            for k in range(1, N_FULL - 1):
                nc.vector.tensor_add(acc[:], acc[:], full_t[k][:])
            nc.vector.tensor_add(acc[0:C_P], acc[0:C_P], tc_[0:C_P])
            # Fold acc into PSUM (runs while the last slot streams in),
            # then the PE takes the final slot directly: the tail after
            # the last input byte is sem + 4 passes + copy.
            mm(acc, 0, P, start=False, stop=False)
            mm(full_t[N_FULL - 1], 0, P, start=False, stop=True)

            # PSUM -> SBUF in parallel halves (DVE + ACT).
            bc = bcast_pool.tile([P, D], f32)
            nc.vector.tensor_copy(bc[:, 0:H], ps[:, 0:H])
            nc.scalar.copy(bc[:, H:D], ps[:, H:D])

            # ---- output DMAs (scalar HWDGE queue), broadcast source,
            # same slow-engine row split as the input.
            o_a = of[0 : N_FULL * P].rearrange(
                "(i n p) m -> i p n m", i=N_WA, n=W_REP, p=P
            )
            src_a = bc[:].unsqueeze(1).to_broadcast((P, W_REP, D))
            for i in range(N_WA):
                nc.scalar.dma_start(o_a[i], src_a)
            o_b1 = of[N_FULL * P : N_FULL * P + B1_N * B1_P].rearrange(
                "(n p) m -> p n m", p=B1_P
            )
            nc.scalar.dma_start(
                o_b1, bc[0:B1_P].unsqueeze(1).to_broadcast((B1_P, B1_N, D))
            )
            nb2 = B2_P1 - B2_P0
            r0 = N_FULL * P + B1_N * B1_P
            o_b2 = of[r0 : r0 + B2_N * nb2].rearrange("(n p) m -> p n m", p=nb2)
            nc.scalar.dma_start(
                o_b2, bc[B2_P0:B2_P1].unsqueeze(1).to_broadcast((nb2, B2_N, D))
            )
            nc.scalar.dma_start(of[S - C_P : S], bc[0:C_P])

    nc.compile()
    return nc


def _get_program():
    if "nc" not in _CACHE:
        _CACHE["nc"] = _build_program()
    return _CACHE["nc"]


def kernel(query: np.ndarray, value: np.ndarray) -> np.ndarray:
    from concourse.bass_utils import run_bass_kernel_spmd

    del query  # output is exactly independent of query (see module docstring)
    value = np.ascontiguousarray(value, dtype=np.float32)
    assert value.shape == (B, S, D)

    nc = _get_program()
    in_maps = [{"value": value[b]} for b in range(B)]
    try:
        res = run_bass_kernel_spmd(nc, in_maps, list(range(B)))
    except Exception:
        # The tunneled runtime occasionally surfaces a transient
        # NRT_EXEC_UNIT_UNRECOVERABLE on the first dispatch; retry once.
        import time

        time.sleep(2.0)
        res = run_bass_kernel_spmd(nc, in_maps, list(range(B)))
    return np.stack([res.results[b]["out"] for b in range(B)], axis=0)
